# revision 12
# baseline (speedup 1.0000x reference)
"""nn_Attention_50749333569807 Bass/Tile kernel for 8 Trainium2 NeuronCores.

Model: x -> 1x1 conv (qkv) -> depthwise 3x3 -> channel attention
(L2-normalized q,k over spatial, softmax over key channels) -> 1x1 proj.

Sharding: 8 cores = 4 batches x 2 spatial halves (64 image rows each).
Each core computes its half end-to-end in bf16; the channel Gram
matrices and row sum-of-squares (contractions over the full spatial
axis) are completed with a tiny pairwise AllReduce (~230KB). Halo rows
come from the host (zeros at image borders, neighbor rows at the half
boundary), so SAME padding is uniform across cores.

Per-core pipeline (bf16 GEMMs, fp32 PSUM accumulation):
  1. qkv GEMM over 66 local rows, chunk-pipelined on PE; ACT evacuation.
  2. depthwise 3x3 = 9 accumulating passes with diagonal weight blocks;
     PE taps are diag-matmuls over shifted access patterns (per-image-row
     matmuls give SAME borders for free); optional DVE taps use the fused
     scalar_tensor_tensor AXPY into a bf16 chain merged at evacuation.
  3. q,k: xbar DMA-transpose to [n, c] chunks; Gram accumulated in PSUM
     over ctile pairs (+4 small straddle blocks for heads 2/5); sumsq on
     ACT (Square + accum_out). v: bounced to DRAM in (h, h+4) head-pair
     row layout for attn@v.
  4. AllReduce(add) of the raw Gram/sumsq tile between half-pairs.
  5. Norms (ACT sqrt + one Newton step + eps clamp + DVE reciprocal),
     outer products with temperature folded in, softmax over key
     channels, per-head PE transposes, block-diag attn@v (pair-packed
     K=128), projection with host-permuted W_proj, bf16 output.
"""

import contextlib
import numpy as np
import ml_dtypes

B, DIM, H, W = 4, 384, 128, 128
HEADS = 8
CH = DIM // HEADS  # 48
QKV = 3 * DIM  # 1152
ROWS_LOC = 66
NLOC = ROWS_LOC * W  # 8448
NOWN = 64 * W  # 8192
N_CORES = 8
GCHUNKS = 17  # 16 x 4 rows + 1 x 2 rows
OCHUNKS = 16
EPS = 1e-12

# tap order: center first (covers the full PSUM tile before partial taps)
TAPS = [(0, 0), (-1, -1), (-1, 0), (-1, 1), (0, -1), (0, 1), (1, -1), (1, 0), (1, 1)]
TAP_ENGINE = ["pe"] * 9  # "pe" or "dve" per tap

_CACHE = {}
DEBUG = False


def _gemm_chunk_cols(g):
    return 512 if g < 16 else 256


def split_multiwaits(nc, max_waits=1):
    """This container's walrus accepts a single sync-wait per instruction;
    split extras into single-wait NOPs placed before the instruction."""
    import concourse.mybir as mybir

    for f in nc.m.functions:
        for bb in f.blocks:
            insts = bb.instructions
            out = []
            for ins in insts:
                si = ins.sync_info
                if si is not None and si.on_wait and len(si.on_wait) > max_waits:
                    waits = list(si.on_wait)
                    for w in waits[:-max_waits]:
                        nop = mybir.InstNoOp(
                            name=nc.get_next_instruction_name(),
                            engine=ins.engine,
                            ins=[], outs=[],
                            sync_info=mybir.SyncInfo(on_wait=[w], on_update=[]),
                        )
                        out.append(nop)
                    ins.sync_info = mybir.SyncInfo(
                        on_wait=waits[-max_waits:], on_update=list(si.on_update)
                    )
                out.append(ins)
            insts[:] = out


def _build_nc():
    import concourse.bass as bass
    import concourse.mybir as mb
    import concourse.tile as tile
    from concourse.masks import make_identity

    f32 = mb.dt.float32
    bf16 = mb.dt.bfloat16
    Act = mb.ActivationFunctionType
    Alu = mb.AluOpType

    nc = bass.Bass("TRN2", num_devices=N_CORES)

    xl_d = nc.declare_dram_parameter("xl", [DIM, NLOC], bf16, isOutput=False)
    wqkvT_d = nc.declare_dram_parameter("wqkvT", [DIM, QKV], bf16, isOutput=False)
    wdiag_d = nc.declare_dram_parameter("wdiag", [128, 81 * 128], bf16, isOutput=False)
    wdwps_d = nc.declare_dram_parameter("wdwps", [128, 81], f32, isOutput=False)
    wprojP_d = nc.declare_dram_parameter("wprojP", [512, DIM], bf16, isOutput=False)
    tmprow_d = nc.declare_dram_parameter("tmprow", [1, DIM], f32, isOutput=False)
    yout_d = nc.declare_dram_parameter("yout", [DIM, NOWN], bf16, isOutput=True)
    if DEBUG:
        dbg_pre = nc.declare_dram_parameter("dbg_pre", [128, 512], bf16, isOutput=True)
        dbg_dw = nc.declare_dram_parameter("dbg_dw", [128, 512], bf16, isOutput=True)
        dbg_dwv = nc.declare_dram_parameter("dbg_dwv", [128, 512], bf16, isOutput=True)
        dbg_qkt = nc.declare_dram_parameter("dbg_qkt", [128, 512], bf16, isOutput=True)
        dbg_gram = nc.declare_dram_parameter("dbg_gram", [128, 488], f32, isOutput=True)
        dbg_ar = nc.declare_dram_parameter("dbg_ar", [128, 488], f32, isOutput=True)
        dbg_attn = nc.declare_dram_parameter("dbg_attn", [48, 384], f32, isOutput=True)
        dbg_ao = nc.declare_dram_parameter("dbg_ao", [128, 512], bf16, isOutput=True)
        dbg_rinv = nc.declare_dram_parameter("dbg_rinv", [128, 6], f32, isOutput=True)
        dbg_rrow = nc.declare_dram_parameter("dbg_rrow", [1, 768], f32, isOutput=True)
        dbg_op = nc.declare_dram_parameter("dbg_op", [48, 384], f32, isOutput=True)
        dbg_smin = nc.declare_dram_parameter("dbg_smin", [48, 384], f32, isOutput=True)

    pe_taps = [i for i, e in enumerate(TAP_ENGINE) if e == "pe"]
    dve_taps = [i for i, e in enumerate(TAP_ENGINE) if e == "dve"]
    assert TAP_ENGINE[0] == "pe", "center tap must be PE (PSUM init)"

    def rloc(r):  # local row -> (gemm chunk, row-within-chunk)
        return r // 4, r % 4

    with tile.TileContext(nc) as tc, contextlib.ExitStack() as ctx:
        singles = ctx.enter_context(tc.tile_pool(name="singles", bufs=1))
        xpool = ctx.enter_context(tc.tile_pool(name="xpool", bufs=4))
        prep = ctx.enter_context(tc.tile_pool(name="prep", bufs=4))
        dwp = ctx.enter_context(tc.tile_pool(name="dwp", bufs=3))
        qkTp = ctx.enter_context(tc.tile_pool(name="qkTp", bufs=3))
        chainp = ctx.enter_context(tc.tile_pool(name="chainp", bufs=3))
        smalls = ctx.enter_context(tc.tile_pool(name="smalls", bufs=1))
        vrd = ctx.enter_context(tc.tile_pool(name="vrd", bufs=3))
        aop = ctx.enter_context(tc.tile_pool(name="aop", bufs=2))
        outp = ctx.enter_context(tc.tile_pool(name="outp", bufs=3))
        dramp = ctx.enter_context(tc.tile_pool(name="dramp", bufs=1, space="DRAM"))

        vdw_t = dramp.tile([4, 128, NOWN], bf16, name="vdw")
        cc_in = dramp.tile([128, 488], f32, name="cc_in")
        cc_out = dramp.tile([128, 488], f32, name="cc_out")

        # ---- persistent loads ----
        wq_sb = []
        for kc in range(3):
            t = singles.tile([128, QKV], bf16, name=f"wq{kc}")
            nc.sync.dma_start(out=t, in_=wqkvT_d.ap()[kc * 128:(kc + 1) * 128, :])
            wq_sb.append(t)
        wdiag_sb = singles.tile([128, 81 * 128], bf16, name="wdiag")
        nc.sync.dma_start(out=wdiag_sb, in_=wdiag_d.ap())
        wdwps_sb = singles.tile([128, 81], f32, name="wdwps")
        nc.sync.dma_start(out=wdwps_sb, in_=wdwps_d.ap())
        wproj_sb = []
        for pk in range(4):
            t = singles.tile([128, DIM], bf16, name=f"wproj{pk}")
            nc.sync.dma_start(out=t, in_=wprojP_d.ap()[pk * 128:(pk + 1) * 128, :])
            wproj_sb.append(t)
        tmprow_sb = singles.tile([1, DIM], f32, name="tmprow")
        nc.sync.dma_start(out=tmprow_sb, in_=tmprow_d.ap())
        ident_bf = singles.tile([128, 128], bf16, name="ident_bf")
        make_identity(nc, ident_bf)
        ident_f32 = singles.tile([128, 128], f32, name="ident_f32")
        make_identity(nc, ident_f32)

        ssq_slots = singles.tile([128, 6, OCHUNKS], f32, name="ssq_slots")
        sq_scratch = singles.tile([128, 512], f32, name="sq_scratch")

        # zero the unused rows of the v pair-layout scratch
        zsb = singles.tile([128, 2048], bf16, name="zsb")
        nc.vector.memset(zsb, 0.0)
        for hp in range(4):
            for c0 in range(0, NOWN, 2048):
                nc.sync.dma_start(out=vdw_t[hp, 48:64, c0:c0 + 2048], in_=zsb[0:16, :])
                nc.sync.dma_start(out=vdw_t[hp, 112:128, c0:c0 + 2048], in_=zsb[0:16, :])

        x_tiles = {}
        pre_tiles = {}
        qkT_tiles = {}

        def load_x(g):
            ncols = _gemm_chunk_cols(g)
            for kc in range(3):
                t = xpool.tile([128, 512], bf16, tag=f"x{kc}", name=f"xt{kc}_{g}")
                nc.sync.dma_start(
                    out=t[:, :ncols],
                    in_=xl_d.ap()[kc * 128:(kc + 1) * 128, g * 512:g * 512 + ncols],
                )
                x_tiles[(kc, g)] = t

        def v_out_runs(ct):
            """[(src_row0, nrows, pair, dst_row0)] for v ctile ct (0..2)."""
            runs, r = [], 0
            base = ct * 128
            while r < 128:
                c = base + r
                head, off = c // CH, c % CH
                hp, half = head % 4, head // 4
                n = min(CH - off, 128 - r)
                runs.append((r, n, hp, half * 64 + off))
                r += n
            return runs

        with contextlib.ExitStack() as psctx:
            ps_gemm = psctx.enter_context(tc.tile_pool(name="ps_gemm", bufs=3, space="PSUM"))
            ps_dw = psctx.enter_context(tc.tile_pool(name="ps_dw", bufs=2, space="PSUM"))
            ps_gram = psctx.enter_context(tc.tile_pool(name="ps_gram", bufs=1, space="PSUM"))

            gram_ps = ps_gram.tile([128, 480], f32, name="gram_ps")
            # straddle slots: s0=(0,1)[32x16]@384, s1=(1,0)[16x32]@400,
            #                 s2=(1,2)[16x32]@432, s3=(2,1)[32x16]@464

            def gemm_chunk(g):
                ncols = _gemm_chunk_cols(g)
                for ot in range(9):
                    ps = ps_gemm.tile([128, 512], f32, tag="gemm", name=f"gps{ot}_{g}")
                    for kc in range(3):
                        nc.tensor.matmul(
                            ps[:, :ncols],
                            wq_sb[kc][:, ot * 128:(ot + 1) * 128],
                            x_tiles[(kc, g)][:, :ncols],
                            start=(kc == 0), stop=(kc == 2),
                        )
                    pre = prep.tile([128, 512], bf16, tag=f"pre{ot}", name=f"pre{ot}_{g}")
                    nc.scalar.copy(pre[:, :ncols], ps[:, :ncols])
                    pre_tiles[(ot, g)] = pre
                for kc in range(3):
                    del x_tiles[(kc, g)]

            def dw_chunk(ct, j):
                r0 = 1 + 4 * j
                psd = ps_dw.tile([128, 512], f32, tag="dw", name=f"dps{ct}_{j}")
                n_pe_mms = len(pe_taps) * 4
                k = 0
                for ti in pe_taps:
                    dy, dx = TAPS[ti]
                    blk = (ct * 9 + ti) * 128
                    iw0, iw1 = max(0, dx), 128 + min(0, dx)
                    ow0, ow1 = max(0, -dx), 128 + min(0, -dx)
                    for rr in range(4):
                        g, wsr = rloc(r0 + rr + dy)
                        nc.tensor.matmul(
                            psd[:, rr * 128 + ow0:rr * 128 + ow1],
                            wdiag_sb[:, blk:blk + 128],
                            pre_tiles[(ct, g)][:, wsr * 128 + iw0:wsr * 128 + iw1],
                            start=(k == 0), stop=(k == n_pe_mms - 1),
                        )
                        k += 1
                chain = None
                for kd, ti in enumerate(dve_taps):
                    dy, dx = TAPS[ti]
                    sc = wdwps_sb[:, ct * 9 + ti:ct * 9 + ti + 1]
                    iw0, iw1 = max(0, dx), 128 + min(0, dx)
                    ow0, ow1 = max(0, -dx), 128 + min(0, -dx)
                    if kd == 0:
                        chain = chainp.tile([128, 4, 128], bf16, tag="chain", name=f"ch{ct}_{j}")
                        nc.vector.memset(chain, 0.0)
                    rr = 0
                    while rr < 4:
                        g, wsr = rloc(r0 + rr + dy)
                        nrun = 1
                        while rr + nrun < 4 and rloc(r0 + rr + nrun + dy)[0] == g:
                            nrun += 1
                        src = pre_tiles[(ct, g)].rearrange("p (r w) -> p r w", w=128)[
                            :, wsr:wsr + nrun, iw0:iw1]
                        dst = chain[:, rr:rr + nrun, ow0:ow1]
                        nc.vector.scalar_tensor_tensor(
                            dst, src, sc, dst, op0=Alu.mult, op1=Alu.add,
                        )
                        rr += nrun
                dw = dwp.tile([128, 512], bf16, tag=f"dw{ct}", name=f"dw{ct}_{j}")
                if chain is not None:
                    nc.vector.tensor_add(dw, psd, chain.rearrange("p r w -> p (r w)"))
                else:
                    nc.vector.tensor_copy(dw, psd)
                return dw

            def gram_chunk(j):
                qT = [qkT_tiles[(ct, j)] for ct in range(3)]
                kT = [qkT_tiles[(ct, j)] for ct in range(3, 6)]
                for nb in range(4):
                    st = (j == 0 and nb == 0)
                    fin = (j == OCHUNKS - 1 and nb == 3)
                    mms = [
                        (gram_ps[:, 0:128], qT[0][:, nb, :], kT[0][:, nb, :]),
                        (gram_ps[:, 128:256], qT[1][:, nb, :], kT[1][:, nb, :]),
                        (gram_ps[:, 256:384], qT[2][:, nb, :], kT[2][:, nb, :]),
                        (gram_ps[0:32, 384:400], qT[0][:, nb, 96:128], kT[1][:, nb, 0:16]),
                        (gram_ps[0:16, 400:432], qT[1][:, nb, 0:16], kT[0][:, nb, 96:128]),
                        (gram_ps[0:16, 432:464], qT[1][:, nb, 112:128], kT[2][:, nb, 0:32]),
                        (gram_ps[0:32, 464:480], qT[2][:, nb, 0:32], kT[1][:, nb, 112:128]),
                    ]
                    for mi, (o, lt, rt) in enumerate(mms):
                        nc.tensor.matmul(
                            o, lt, rt, start=st, stop=(fin and mi == len(mms) - 1)
                        )
                        st = False

            load_x(0)
            load_x(1)
            gemm_chunk(0)
            for j in range(OCHUNKS):
                if j + 1 < GCHUNKS:
                    if j + 2 < GCHUNKS:
                        load_x(j + 2)
                    gemm_chunk(j + 1)
                if DEBUG and j == 4:
                    nc.sync.dma_start(out=dbg_pre.ap(), in_=pre_tiles[(0, 4)])
                for ct in range(9):
                    dw = dw_chunk(ct, j)
                    if DEBUG and j == 4 and ct == 0:
                        nc.sync.dma_start(out=dbg_dw.ap(), in_=dw)
                    if DEBUG and j == 4 and ct == 6:
                        nc.sync.dma_start(out=dbg_dwv.ap(), in_=dw)
                    if ct < 6:
                        nc.scalar.activation(
                            sq_scratch, dw, func=Act.Square,
                            accum_out=ssq_slots[:, ct, j:j + 1],
                        )
                        qkt = qkTp.tile([128, 4, 128], bf16, tag=f"qkT{ct}",
                                        name=f"qkT{ct}_{j}")
                        nc.sync.dma_start_transpose(out=qkt[:], in_=dw)
                        qkT_tiles[(ct, j)] = qkt
                        if DEBUG and j == 4 and ct == 0:
                            nc.sync.dma_start(
                                out=dbg_qkt.ap().rearrange("p (b c) -> p b c", b=4),
                                in_=qkt[:])
                    else:
                        for (sr, n, hp, dr) in v_out_runs(ct - 6):
                            nc.sync.dma_start(
                                out=vdw_t[hp, dr:dr + n, j * 512:(j + 1) * 512],
                                in_=dw[sr:sr + n, :],
                            )
                gram_chunk(j)
                for ct in range(6):
                    del qkT_tiles[(ct, j)]

            # ---- sumsq finish + ship partials ----
            ssq_sum = smalls.tile([128, 6], f32, name="ssq_sum")
            nc.vector.tensor_reduce(
                ssq_sum, ssq_slots, axis=mb.AxisListType.X, op=Alu.add
            )
            gram_sb = smalls.tile([128, 488], f32, name="gram_sb")
            nc.vector.tensor_copy(gram_sb[:, 0:480], gram_ps[:, 0:480])
            nc.vector.tensor_copy(gram_sb[:, 480:486], ssq_sum)
            nc.vector.memset(gram_sb[:, 486:488], 0.0)
            nc.sync.dma_start(out=cc_in, in_=gram_sb)
            if DEBUG:
                nc.sync.dma_start(out=dbg_gram.ap(), in_=gram_sb)

        nc.gpsimd.collective_compute(
            "AllReduce",
            Alu.add,
            replica_groups=[[0, 1], [2, 3], [4, 5], [6, 7]],
            ins=[cc_in],
            outs=[cc_out],
        )
        ar_sb = smalls.tile([128, 488], f32, name="ar_sb")
        nc.sync.dma_start(out=ar_sb, in_=cc_out)
        if DEBUG:
            nc.sync.dma_start(out=dbg_ar.ap(), in_=ar_sb)

        with contextlib.ExitStack() as psctx2:
            ps_sm = psctx2.enter_context(tc.tile_pool(name="ps_sm", bufs=1, space="PSUM"))
            ps_mm = psctx2.enter_context(tc.tile_pool(name="ps_mm", bufs=2, space="PSUM"))

            # ---- norms ----
            ssq_all = ar_sb[:, 480:486]
            n0 = smalls.tile([128, 6], f32, name="n0")
            nc.scalar.activation(n0, ssq_all, func=Act.Sqrt)
            rn0 = smalls.tile([128, 6], f32, name="rn0")
            nc.vector.reciprocal(rn0, n0)
            n1 = smalls.tile([128, 6], f32, name="n1")
            nc.vector.tensor_mul(n1, ssq_all, rn0)
            nc.vector.tensor_add(n1, n1, n0)
            nc.vector.tensor_scalar(n1, n1, 0.5, EPS, op0=Alu.mult, op1=Alu.max)
            rinv = smalls.tile([128, 6], f32, name="rinv")
            nc.vector.reciprocal(rinv, n1)
            if DEBUG:
                nc.sync.dma_start(out=dbg_rinv.ap(), in_=rinv)

            rr_ps = ps_sm.tile([1, 768], f32, tag="sm", name="rr_ps")
            for ctt in range(6):
                nc.tensor.matmul(
                    rr_ps[0:1, ctt * 128:(ctt + 1) * 128],
                    rinv[:, ctt:ctt + 1], ident_f32,
                    start=(ctt in (0, 4)), stop=(ctt in (3, 5)),
                )
            rrow = smalls.tile([1, 768], f32, name="rrow")
            nc.vector.tensor_copy(rrow, rr_ps)
            nc.vector.tensor_mul(rrow[0:1, 0:DIM], rrow[0:1, 0:DIM], tmprow_sb)
            if DEBUG:
                nc.sync.dma_start(out=dbg_rrow.ap(), in_=rrow)

            op_ps = ps_sm.tile([48, HEADS * CH], f32, tag="sm", name="op_ps")
            for h in range(HEADS):
                nc.tensor.matmul(
                    op_ps[:, h * CH:(h + 1) * CH],
                    rrow[0:1, h * CH:(h + 1) * CH],
                    rrow[0:1, DIM + h * CH:DIM + (h + 1) * CH],
                    start=(h == 0), stop=(h == HEADS - 1),
                )
            op_sb = smalls.tile([48, HEADS * CH], f32, name="op_sb")
            nc.vector.tensor_copy(op_sb, op_ps)
            if DEBUG:
                nc.sync.dma_start(out=dbg_op.ap(), in_=op_sb)

            # ---- per-head gram gather ----
            sm_in = smalls.tile([48, HEADS * CH], f32, name="sm_in")
            for h in range(HEADS):
                i, o = (CH * h) // 128, (CH * h) % 128
                if o + CH <= 128:
                    nc.sync.dma_start(
                        out=sm_in[:, h * CH:(h + 1) * CH],
                        in_=ar_sb[o:o + CH, i * 128 + o:i * 128 + o + CH],
                    )
                elif h == 2:
                    nc.sync.dma_start(out=sm_in[0:32, h * CH:h * CH + 32],
                                      in_=ar_sb[96:128, 96:128])
                    nc.sync.dma_start(out=sm_in[0:32, h * CH + 32:h * CH + 48],
                                      in_=ar_sb[0:32, 384:400])
                    nc.sync.dma_start(out=sm_in[32:48, h * CH:h * CH + 32],
                                      in_=ar_sb[0:16, 400:432])
                    nc.sync.dma_start(out=sm_in[32:48, h * CH + 32:h * CH + 48],
                                      in_=ar_sb[0:16, 128:144])
                else:  # h == 5
                    nc.sync.dma_start(out=sm_in[0:16, h * CH:h * CH + 16],
                                      in_=ar_sb[112:128, 240:256])
                    nc.sync.dma_start(out=sm_in[0:16, h * CH + 16:h * CH + 48],
                                      in_=ar_sb[0:16, 432:464])
                    nc.sync.dma_start(out=sm_in[16:48, h * CH:h * CH + 16],
                                      in_=ar_sb[0:32, 464:480])
                    nc.sync.dma_start(out=sm_in[16:48, h * CH + 16:h * CH + 48],
                                      in_=ar_sb[0:32, 256:288])

            if DEBUG:
                nc.sync.dma_start(out=dbg_smin.ap(), in_=sm_in)
            # ---- softmax ----
            lg = smalls.tile([48, HEADS * CH], f32, name="lg")
            nc.vector.tensor_mul(lg, sm_in, op_sb)
            mx = smalls.tile([48, HEADS], f32, name="mx")
            nc.vector.tensor_reduce(
                mx, lg.rearrange("p (h c) -> p h c", h=HEADS),
                axis=mb.AxisListType.X, op=Alu.max,
            )
            for h in range(HEADS):
                nc.vector.tensor_scalar(
                    lg[:, h * CH:(h + 1) * CH], lg[:, h * CH:(h + 1) * CH],
                    mx[:, h:h + 1], None, op0=Alu.subtract,
                )
            nc.scalar.activation(lg, lg, func=Act.Exp)
            sm_sum = smalls.tile([48, HEADS], f32, name="sm_sum")
            nc.vector.tensor_reduce(
                sm_sum, lg.rearrange("p (h c) -> p h c", h=HEADS),
                axis=mb.AxisListType.X, op=Alu.add,
            )
            rsum = smalls.tile([48, HEADS], f32, name="rsum")
            nc.vector.reciprocal(rsum, sm_sum)
            attn = smalls.tile([48, HEADS * CH], bf16, name="attn")
            for h in range(HEADS):
                nc.vector.tensor_scalar(
                    attn[:, h * CH:(h + 1) * CH], lg[:, h * CH:(h + 1) * CH],
                    rsum[:, h:h + 1], None, op0=Alu.mult,
                )

            at_ps = ps_sm.tile([48, HEADS * CH], bf16, tag="sm", name="at_ps")
            for h in range(HEADS):
                nc.tensor.matmul(
                    at_ps[:, h * CH:(h + 1) * CH],
                    attn[:, h * CH:(h + 1) * CH], ident_bf[0:48, 0:48],
                    is_transpose=True, start=(h == 0), stop=(h == HEADS - 1),
                )
            if DEBUG:
                dbg_at = smalls.tile([48, 384], f32, name="dbg_at")
                nc.vector.tensor_copy(dbg_at, attn)
                nc.sync.dma_start(out=dbg_attn.ap(), in_=dbg_at)
            attnT = smalls.tile([48, HEADS * CH], bf16, name="attnT")
            nc.vector.tensor_copy(attnT, at_ps)

            bd = []
            for hp in range(4):
                t = smalls.tile([128, 128], bf16, name=f"bd{hp}")
                nc.vector.memset(t, 0.0)
                nc.sync.dma_start(out=t[0:48, 0:48],
                                  in_=attnT[:, hp * CH:(hp + 1) * CH])
                nc.sync.dma_start(out=t[64:112, 64:112],
                                  in_=attnT[:, (hp + 4) * CH:(hp + 5) * CH])
                bd.append(t)

            # ---- attn@v + projection ----
            for j in range(OCHUNKS):
                ao = []
                for hp in range(4):
                    vt = vrd.tile([128, 512], bf16, tag=f"v{hp}", name=f"vt{hp}_{j}")
                    nc.sync.dma_start(out=vt, in_=vdw_t[hp, :, j * 512:(j + 1) * 512])
                    pv = ps_mm.tile([128, 512], f32, tag="av", name=f"avps{hp}_{j}")
                    nc.tensor.matmul(pv, bd[hp], vt, start=True, stop=True)
                    at = aop.tile([128, 512], bf16, tag=f"ao{hp}", name=f"ao{hp}_{j}")
                    nc.scalar.copy(at, pv)
                    if DEBUG and j == 4 and hp == 0:
                        nc.sync.dma_start(out=dbg_ao.ap(), in_=at)
                    ao.append(at)
                for mt in range(3):
                    pp = ps_mm.tile([128, 512], f32, tag="proj", name=f"pps{mt}_{j}")
                    for pk in range(4):
                        nc.tensor.matmul(
                            pp, wproj_sb[pk][:, mt * 128:(mt + 1) * 128], ao[pk],
                            start=(pk == 0), stop=(pk == 3),
                        )
                    ot = outp.tile([128, 512], bf16, tag=f"out{mt}", name=f"ot{mt}_{j}")
                    nc.scalar.copy(ot, pp)
                    nc.sync.dma_start(
                        out=yout_d.ap()[mt * 128:(mt + 1) * 128,
                                        j * 512:(j + 1) * 512],
                        in_=ot,
                    )
    return nc


def _prepare_shared_weights(w_qkv, w_dw, w_proj, temperature):
    bf = ml_dtypes.bfloat16
    w_qkv = np.asarray(w_qkv, np.float32)
    w_dw = np.asarray(w_dw, np.float32).reshape(QKV, 9)  # tap idx = 3*(dy+1)+(dx+1)
    w_proj = np.asarray(w_proj, np.float32)
    temp = np.asarray(temperature, np.float32).reshape(HEADS)

    wqkvT = np.ascontiguousarray(w_qkv.T).astype(bf)

    wdiag = np.zeros((128, 81 * 128), np.float32)
    wdwps = np.zeros((128, 81), np.float32)
    for ct in range(9):
        for ti, (dy, dx) in enumerate(TAPS):
            tap = 3 * (dy + 1) + (dx + 1)
            wv = w_dw[ct * 128:(ct + 1) * 128, tap]
            blk = (ct * 9 + ti) * 128
            wdiag[np.arange(128), blk + np.arange(128)] = wv
            wdwps[:, ct * 9 + ti] = wv
    wdiag = wdiag.astype(bf)

    wprojT = w_proj.T  # [c, o]
    wprojP = np.zeros((512, DIM), np.float32)
    for pk in range(4):
        wprojP[pk * 128 + 0:pk * 128 + 48] = wprojT[CH * pk:CH * pk + CH]
        wprojP[pk * 128 + 64:pk * 128 + 112] = wprojT[CH * (pk + 4):CH * (pk + 4) + CH]
    wprojP = wprojP.astype(bf)

    tmprow = np.repeat(temp, CH).reshape(1, DIM).astype(np.float32)
    return {
        "wqkvT": wqkvT, "wdiag": wdiag, "wdwps": wdwps.astype(np.float32),
        "wprojP": wprojP, "tmprow": tmprow,
    }


def _make_in_maps(x, w_qkv, w_dw, w_proj, temperature):
    bf = ml_dtypes.bfloat16
    x = np.asarray(x, np.float32)
    shared = _prepare_shared_weights(w_qkv, w_dw, w_proj, temperature)
    in_maps = []
    for core in range(N_CORES):
        b, half = core // 2, core % 2
        h0 = half * 64
        xp = np.zeros((DIM, ROWS_LOC, W), np.float32)
        lo, hi = h0 - 1, h0 + 65
        slo, shi = max(lo, 0), min(hi, H)
        xp[:, slo - lo:shi - lo, :] = x[b, :, slo:shi, :]
        in_maps.append({"xl": xp.reshape(DIM, NLOC).astype(bf), **shared})
    return in_maps


def _get_nc():
    if "nc" not in _CACHE:
        nc = _build_nc()
        split_multiwaits(nc)
        _CACHE["nc"] = nc
    return _CACHE["nc"]


def _assemble(results):
    out = np.empty((B, DIM, H, W), np.float32)
    for core in range(N_CORES):
        b, half = core // 2, core % 2
        y = results[core]["yout"].astype(np.float32).reshape(DIM, 64, W)
        out[b, :, half * 64:half * 64 + 64, :] = y
    return out


def kernel(x, w_qkv, w_dw, w_proj, temperature):
    from concourse.bass_utils import run_bass_kernel_spmd

    in_maps = _make_in_maps(x, w_qkv, w_dw, w_proj, temperature)
    nc = _get_nc()
    res = run_bass_kernel_spmd(nc, in_maps, core_ids=list(range(N_CORES)))
    _CACHE["last_results"] = res
    return _assemble(res.results)


def benchmark(x, w_qkv, w_dw, w_proj, temperature, iters=6):
    """Compile once, execute `iters` times with device-resident inputs.
    Returns (output ndarray, per-iteration wall times in ns)."""
    import time
    import jax
    import jax.numpy as jnp
    from jax.sharding import Mesh, PartitionSpec, NamedSharding
    from jax.experimental.shard_map import shard_map
    from concourse import bass2jax
    from concourse.bass2jax import _bass_exec_p, install_neuronx_cc_hook
    import concourse.mybir as mb

    install_neuronx_cc_hook()
    in_maps = _make_in_maps(x, w_qkv, w_dw, w_proj, temperature)
    nc = _get_nc()

    in_names, out_names, out_avals = [], [], []
    for alloc in nc.m.functions[0].allocations:
        if not isinstance(alloc, mb.MemoryLocationSet):
            continue
        name = alloc.memorylocations[0].name
        if alloc.kind == "ExternalInput":
            if nc.partition_id_tensor is None or name != nc.partition_id_tensor.name:
                in_names.append(name)
        elif alloc.kind == "ExternalOutput":
            out_names.append(name)
            out_avals.append(
                jax.core.ShapedArray(tuple(alloc.tensor_shape), mb.dt.np(alloc.dtype))
            )
    n_params = len(in_names)
    zero_outs = [np.zeros(a.shape, a.dtype) for a in out_avals]
    all_in_names = list(in_names) + list(out_names)
    if nc.partition_id_tensor is not None:
        all_in_names.append(nc.partition_id_tensor.name)

    donate = tuple(range(n_params, n_params + len(out_names)))

    def _body(*args):
        operands = list(args)
        if nc.partition_id_tensor is not None:
            operands.append(bass2jax.partition_id_tensor())
        return tuple(
            _bass_exec_p.bind(
                *operands,
                out_avals=tuple(out_avals),
                in_names=tuple(all_in_names),
                out_names=tuple(out_names),
                lowering_input_output_aliases=(),
                sim_require_finite=True,
                sim_require_nnan=True,
                nc=nc,
            )
        )

    devices = jax.devices()[:N_CORES]
    mesh = Mesh(np.asarray(devices), ("core",))
    in_specs = (PartitionSpec("core"),) * (n_params + len(out_names))
    out_specs = (PartitionSpec("core"),) * len(out_names)
    fn = jax.jit(
        shard_map(_body, mesh=mesh, in_specs=in_specs, out_specs=out_specs,
                  check_rep=False),
        donate_argnums=donate, keep_unused=True,
    )

    sh = NamedSharding(mesh, PartitionSpec("core"))
    concat_in = [
        jax.device_put(
            np.concatenate([np.asarray(in_maps[c][n]) for c in range(N_CORES)], 0), sh
        )
        for n in in_names
    ]
    zsets = [
        [jax.device_put(np.zeros((N_CORES * z.shape[0], *z.shape[1:]), z.dtype), sh)
         for z in zero_outs]
        for _ in range(iters)
    ]

    times = []
    out_arrs = None
    for it in range(iters):
        for a in concat_in:
            a.block_until_ready()
        t0 = time.perf_counter_ns()
        res = fn(*concat_in, *zsets[it])
        for r in res:
            r.block_until_ready()
        times.append(time.perf_counter_ns() - t0)
        if it == iters - 1:
            out_arrs = res
    results = [
        {n: np.asarray(out_arrs[i]).reshape(N_CORES, *out_avals[i].shape)[c]
         for i, n in enumerate(out_names)}
        for c in range(N_CORES)
    ]
    return _assemble(results), times
